# revision 20
# baseline (speedup 1.0000x reference)
"""Chamfer distance L1 + argmin (seg) kernel for Trainium2.

Problem: xyz1 [8, 4096, 3], xyz2 [8, 4096, 3] (fp32).
Returns (cd_loss scalar fp32, idx1 [8, 4096] int32) matching:

    d[b,n,m] = ||xyz1[b,n] - xyz2[b,m]||^2  (= sq1 + sq2 - 2*inner)
    dist1 = min_m d ; idx1 = argmin_m d ; dist2 = min_n d
    cd_loss = 0.5 * (mean(sqrt(relu(dist1))) + mean(sqrt(relu(dist2))))

Sharding: data-parallel over batch B=8 across the 8 NeuronCores (one batch
item per core).  Within a core the N axis is tiled into 32 chunks of 128
(PSUM partition dim), the M axis into 2 halves of 2048 (4 PSUM banks each,
double-buffered).

The matrix materialized in PSUM is u = -(sq2 - 2*inner) = 2*inner - sq2
(negated so DVE max8/max_index implement min/argmin):
    dist1 = sq1 - max_m u ; idx1 = argmax_m u
    dist2 = min_n (sq1 + (sq2-2inner)) = -max_n (u - sq1)

Per (chunk, half): PE runs 4 bank-pairs of accumulating matmuls
(K=1 ones x (-sq2), then K=3 (2x)^T x y); DVE then does, on the wide
[128,2048] tile: max8 -> per-half max, max_index -> per-half argmax,
and scalar_tensor_tensor running max of (u - sq1) into rn.
Chunk epilogue combines the two halves (first-occurrence semantics kept
via the half-winner select; max_index returns first match).
Finale: rn [128,4096] is multiplied by -identity on PE 128x128 at a time
(transpose + negate back to d-space); free-dim min gives dist2; relu +
sqrt + accumulate-sum on ACT; cross-partition sum via a ones-matmul.
Host sums the 8 per-core partial sums into the scalar loss.

Toolchain notes: built on Bacc (compile() legalizes the one-wait-per-
instruction TRN2 constraint); tensor_tensor_reduce is avoided (compiles
but faults at runtime on this stack); wide multi-bank PSUM access patterns
for DVE ops were validated on hardware.
"""

import os
import numpy as np

B = 8
N = 4096
M = 4096
P = 128
NCH = N // P  # 32 n-chunks
BANK = 512
HALF = 2048
NHALF = M // HALF  # 2 halves
BPH = HALF // BANK  # 4 banks per half


def _emit(ctx, tc, x1, x2, out_idx, out_sums):
    from concourse import bass
    import concourse.mybir as mybir
    from concourse.masks import make_identity

    nc = tc.nc
    f32 = mybir.dt.float32
    i32 = mybir.dt.int32
    u32 = mybir.dt.uint32
    mn = mybir.AluOpType.min
    mx = mybir.AluOpType.max
    add = mybir.AluOpType.add
    sub = mybir.AluOpType.subtract
    mult = mybir.AluOpType.mult

    const = ctx.enter_context(tc.tile_pool(name="const", bufs=1))
    work = ctx.enter_context(tc.tile_pool(name="work", bufs=2))
    small = ctx.enter_context(tc.tile_pool(name="small", bufs=3))
    psum = ctx.enter_context(tc.tile_pool(name="psum", bufs=2, space="PSUM"))

    # ---------------- persistent tiles ----------------
    xT = const.tile([3, N], f32)  # xyz1^T rows (scaled by +2 in place)
    yT = const.tile([3, M], f32)  # xyz2^T rows
    nsq2v = const.tile([1, M], f32)  # -||y||^2 row
    sq1c = const.tile([P, NCH], f32)  # ||x||^2 in (p, chunk) layout
    xcp = const.tile([P, 3 * NCH], f32)  # xyz1 in (p, chunk, dim) layout
    rn = const.tile([P, M], f32)  # running column-max of u - sq1
    idx_col = const.tile([P, NCH], i32)
    idxf_col = const.tile([P, NCH], f32)
    d1_col = const.tile([P, NCH], f32)
    d2_col = const.tile([P, NCH], f32)
    nident = const.tile([P, P], f32)  # -identity
    ones3 = const.tile([3, 1], f32)
    ones1 = const.tile([1, P], f32)
    ones128 = const.tile([P, 1], f32)

    # ---- phase A: DMAs + constant fills ----
    nc.vector.memset(rn[:], -3.0e38)
    nc.vector.memset(ones3[:], 1.0)
    nc.vector.memset(ones1[:], 1.0)
    nc.gpsimd.memset(ones128[:], 1.0)
    make_identity(nc, nident[:])
    nc.gpsimd.tensor_scalar_mul(nident[:], nident[:], -1.0)
    nc.sync.dma_start(out=xT[:], in_=x1.rearrange("n d -> d n"))
    nc.sync.dma_start(out=yT[:], in_=x2.rearrange("n d -> d n"))
    nc.sync.dma_start(
        out=xcp[:].rearrange("p (c d) -> p c d", d=3),
        in_=x1.rearrange("(c p) d -> p c d", p=P),
    )

    # ---- phase B: squared norms ----
    # sq1 in (p, chunk) layout, all on DVE
    xcp2 = work.tile([P, 3 * NCH], f32, tag="xcp2")
    nc.vector.tensor_mul(xcp2[:], xcp[:], xcp[:])
    nc.vector.tensor_reduce(
        out=sq1c[:],
        in_=xcp2[:].rearrange("p (c d) -> p c d", d=3),
        axis=mybir.AxisListType.X,
        op=add,
    )
    # sq2 row: square on ACT, partition-sum via ones3 matmuls, negated DVE copy
    sqt2 = work.tile([3, M], f32, tag="sqt")
    nc.scalar.square(sqt2[:], yT[:])
    # +2x on ACT as well (same engine => ordered)
    nc.scalar.mul(xT[:], xT[:], 2.0)

    # PE touch 1: absorb the DVE clock; touch 2: absorb the Pool clock
    ps_t = psum.tile([P, HALF], f32, tag="pw")
    nc.tensor.matmul(
        ps_t[0:1, 0:1], ones3[0:1, 0:1], ones3[0:1, 0:1], start=True, stop=True
    )
    ps_t = psum.tile([P, HALF], f32, tag="pw")
    nc.tensor.matmul(
        ps_t[0:1, 0:1], nident[0:1, 0:1], nident[0:1, 0:1], start=True, stop=True
    )

    for h in range(NHALF):
        ps = psum.tile([P, HALF], f32, tag="pw")
        for b in range(BPH):
            m0 = h * HALF + b * BANK
            nc.tensor.matmul(
                ps[0:1, b * BANK : (b + 1) * BANK],
                ones3[:],
                sqt2[:, m0 : m0 + BANK],
                start=True,
                stop=True,
            )
        nc.vector.tensor_scalar_mul(
            nsq2v[:, h * HALF : (h + 1) * HALF], ps[0:1, :], -1.0
        )

    # ---- main loop ----
    for c in range(NCH):
        lw = xT[:, c * P : (c + 1) * P]
        sq1s = sq1c[:, c : c + 1]
        mxh = []
        ivh = []
        for h in range(NHALF):
            pw = psum.tile([P, HALF], f32, tag="pw")
            for b in range(BPH):
                m0 = h * HALF + b * BANK
                sl = slice(b * BANK, (b + 1) * BANK)
                # u = 2*inner - sq2, accumulated in PSUM
                nc.tensor.matmul(
                    pw[:, sl], ones1[:], nsq2v[:, m0 : m0 + BANK],
                    start=True, stop=False,
                )
                nc.tensor.matmul(
                    pw[:, sl], lw, yT[:, m0 : m0 + BANK], start=False, stop=True
                )
            mxt = small.tile([P, 8], f32, tag=f"mx{h}")
            ivt = small.tile([P, 8], u32, tag=f"iv{h}")
            nc.vector.max(out=mxt[:], in_=pw[:])
            nc.vector.max_index(out=ivt[:], in_max=mxt[:], in_values=pw[:])
            # running column-max of u - sq1 (fused per-partition subtract)
            nc.vector.scalar_tensor_tensor(
                out=rn[:, h * HALF : (h + 1) * HALF],
                in0=pw[:],
                scalar=sq1s,
                in1=rn[:, h * HALF : (h + 1) * HALF],
                op0=sub,
                op1=mx,
            )
            mxh.append(mxt)
            ivh.append(ivt)
        # ---- chunk epilogue (tiny [128,1] ops, all DVE) ----
        j0 = small.tile([P, 1], f32, tag="j0")
        j1 = small.tile([P, 1], f32, tag="j1")
        wf = small.tile([P, 1], f32, tag="wf")
        tmp = small.tile([P, 1], f32, tag="tmp")
        gv = small.tile([P, 1], f32, tag="gv")
        nc.vector.tensor_copy(out=j0[:], in_=ivh[0][:, 0:1])
        nc.vector.tensor_copy(out=j1[:], in_=ivh[1][:, 0:1])
        nc.vector.tensor_scalar_add(j1[:], j1[:], float(HALF))
        # winner select: wf = (max0 >= max1); idx = j1 + wf*(j0 - j1)
        nc.vector.tensor_scalar(
            out=wf[:], in0=mxh[0][:, 0:1], scalar1=mxh[1][:, 0:1], scalar2=None,
            op0=mybir.AluOpType.is_ge,
        )
        nc.vector.tensor_sub(tmp[:], j0[:], j1[:])
        nc.vector.tensor_scalar(
            out=tmp[:], in0=tmp[:], scalar1=wf[:], scalar2=None, op0=mult
        )
        nc.vector.tensor_add(idxf_col[:, c : c + 1], tmp[:], j1[:])
        # dist1 = sq1 - max(u)
        nc.vector.tensor_max(gv[:], mxh[0][:, 0:1], mxh[1][:, 0:1])
        nc.vector.tensor_scalar(
            out=d1_col[:, c : c + 1], in0=gv[:], scalar1=-1.0, scalar2=sq1s,
            op0=mult, op1=add,
        )

    # ---- finale: dist2 via negated-transpose matmuls + free-dim min ----
    for t in range(NCH):
        pst = psum.tile([P, HALF], f32, tag="pw")
        nc.tensor.matmul(
            pst[:, 0:P], rn[:, t * P : (t + 1) * P], nident[:], start=True, stop=True
        )
        nc.vector.tensor_reduce(
            out=d2_col[:, t : t + 1],
            in_=pst[:, 0:P],
            axis=mybir.AxisListType.X,
            op=mn,
        )

    nc.vector.tensor_scalar_max(d1_col[:], d1_col[:], 0.0)
    nc.vector.tensor_scalar_max(d2_col[:], d2_col[:], 0.0)
    nc.vector.tensor_copy(out=idx_col[:], in_=idxf_col[:])

    s12 = small.tile([P, 2], f32, tag="s12")
    rt1 = work.tile([P, NCH], f32, tag="rt")
    rt2 = work.tile([P, NCH], f32, tag="rt")
    nc.scalar.activation(
        out=rt1[:],
        in_=d1_col[:],
        func=mybir.ActivationFunctionType.Sqrt,
        accum_out=s12[:, 0:1],
    )
    nc.scalar.activation(
        out=rt2[:],
        in_=d2_col[:],
        func=mybir.ActivationFunctionType.Sqrt,
        accum_out=s12[:, 1:2],
    )

    # ---- cross-partition sum + stores ----
    ps_s = psum.tile([P, HALF], f32, tag="pw")
    nc.tensor.matmul(ps_s[0:2, 0:1], s12[:], ones128[:], start=True, stop=True)
    sums_sb = small.tile([2, 1], f32, tag="sums")
    nc.vector.tensor_copy(out=sums_sb[:], in_=ps_s[0:2, 0:1])

    nc.sync.dma_start(out=out_sums.rearrange("(p f) -> p f", f=1), in_=sums_sb[:])
    nc.sync.dma_start(out=out_idx.rearrange("(c p) -> p c", p=P), in_=idx_col[:])


def build_program():
    from concourse import bacc, tile
    import concourse.mybir as mybir
    from contextlib import ExitStack

    # Bacc (not raw Bass): its compile() legalizes sync waits (TRN2 allows
    # only one wait per instruction) and lowers the custom DVE ISA ops.
    nc = bacc.Bacc("TRN2", target_bir_lowering=False, debug=False, num_devices=B)
    x1 = nc.dram_tensor("xyz1", [N, 3], mybir.dt.float32, kind="ExternalInput").ap()
    x2 = nc.dram_tensor("xyz2", [M, 3], mybir.dt.float32, kind="ExternalInput").ap()
    out_idx = nc.dram_tensor("out_idx", [N], mybir.dt.int32, kind="ExternalOutput").ap()
    out_sums = nc.dram_tensor(
        "out_sums", [2], mybir.dt.float32, kind="ExternalOutput"
    ).ap()

    with tile.TileContext(nc) as tc:
        with ExitStack() as ctx:
            _emit(ctx, tc, x1, x2, out_idx, out_sums)
    nc.compile()
    return nc


_NC_CACHE = []


def _get_program():
    if not _NC_CACHE:
        _NC_CACHE.append(build_program())
    return _NC_CACHE[0]


def kernel(xyz1, xyz2, **_unused):
    from concourse.bass_utils import run_bass_kernel_spmd

    xyz1 = np.ascontiguousarray(np.asarray(xyz1, dtype=np.float32))
    xyz2 = np.ascontiguousarray(np.asarray(xyz2, dtype=np.float32))
    assert xyz1.shape == (B, N, 3) and xyz2.shape == (B, M, 3)

    nc = _get_program()
    in_maps = [{"xyz1": xyz1[b], "xyz2": xyz2[b]} for b in range(B)]
    res = run_bass_kernel_spmd(nc, in_maps, core_ids=list(range(B)))

    idx = np.stack([res.results[b]["out_idx"] for b in range(B)]).astype(np.int32)
    sums = np.stack([res.results[b]["out_sums"] for b in range(B)]).astype(np.float32)
    denom = np.float32(B * N)
    loss = np.float32(0.5) * (
        sums[:, 0].sum(dtype=np.float32) / denom
        + sums[:, 1].sum(dtype=np.float32) / denom
    )
    return np.float32(loss), idx
